# revision 17
# baseline (speedup 1.0000x reference)
"""MinLSTM cell for Trainium2 (Bass/Tile), data-parallel over batch on 8 cores.

Per core (one batch row):
  - xT [D,T] layout in SBUF; three projections computed as W^T.T @ xT -> [H,T]
    with fp32r matmuls (full PE rate, ~1.5e-4 rel err) accumulating K=768 in PSUM.
  - division-free gates: with Ef = e^{-zf}, Ei = e^{-zi} (ScalarE Exp straight
    from PSUM, bias fused), a = f/(f+i) = (1+Ei)/(2+Ef+Ei) and the reciprocal
    of s2 = 2+Ef+Ei is Exp(-Ln(s2)) — all ACT funcs from one LUT table.
  - the linear recurrence h_t = a_t*h_{t-1} + b_t runs as one VectorE
    tensor_tensor_scan per [128,TC] tile along the free (time) dim, chained
    across T-chunks via the last column of the previous chunk's output.
  - output written as hT [H,T]; host transposes back to [T,H].
"""

import sys

if "/opt/trn_rl_repo" not in sys.path:
    sys.path.insert(0, "/opt/trn_rl_repo")

import numpy as np

B, T, D, H = 8, 4096, 768, 768
TC = 512                    # time-chunk (one PSUM bank of fp32)
NT = T // TC                # 8 chunks
KD = D // 128               # 6 contraction tiles
MH = H // 128               # 6 hidden tiles

_state = {}


def _build():
    import concourse.mybir as mybir
    import concourse.tile as tile
    from concourse import bacc

    f32, f32r = mybir.dt.float32, mybir.dt.float32r
    A = mybir.AluOpType
    Act = mybir.ActivationFunctionType

    nc = bacc.Bacc("TRN2", target_bir_lowering=False, debug=False, num_devices=B)

    xT = nc.dram_tensor("xT", [D, T], f32r, kind="ExternalInput")
    w_d = {p: nc.dram_tensor(f"w{p}", [D, H], f32r, kind="ExternalInput") for p in "fih"}
    b_d = {p: nc.dram_tensor(f"b{p}", [128, MH], f32, kind="ExternalInput") for p in "fih"}
    h0_d = nc.dram_tensor("h0c", [128, MH], f32, kind="ExternalInput")
    hT = nc.dram_tensor("hT", [H, T], f32, kind="ExternalOutput")

    with tile.TileContext(nc) as tc:
        with (
            tc.tile_pool(name="wpool", bufs=1) as wpool,
            tc.tile_pool(name="cpool", bufs=1) as cpool,
            tc.tile_pool(name="xpool", bufs=2) as xpool,
            tc.tile_pool(name="pspool", bufs=8, space="PSUM") as pspool,
            tc.tile_pool(name="wk", bufs=4) as wk,
            tc.tile_pool(name="hpool", bufs=3) as hpool,
        ):
            # Chunk-0 x first on sync, then each projection's weights on its
            # own issue queue so all three arrive in parallel (~7us head).
            xs0 = []
            for kd in range(KD):
                xt = xpool.tile([128, TC], f32r, tag=f"x{kd}", name=f"x0_{kd}")
                nc.sync.dma_start(xt[:], xT[kd * 128:(kd + 1) * 128, 0:TC])
                xs0.append(xt)
            w_q = {"f": nc.gpsimd, "i": nc.scalar, "h": nc.sync}
            w_sb = {p: [] for p in "fih"}
            for p in "fih":
                for kd in range(KD):
                    t = wpool.tile([128, H], f32r, tag=f"w{p}{kd}", name=f"w{p}{kd}")
                    w_q[p].dma_start(t[:], w_d[p][kd * 128:(kd + 1) * 128, :])
                    w_sb[p].append(t)
            b_sb = {}
            for p in "fih":
                b_sb[p] = cpool.tile([128, MH], f32, tag=f"b{p}", name=f"bs{p}")
                nc.gpsimd.dma_start(b_sb[p][:], b_d[p][:])
            h0_sb = cpool.tile([128, MH], f32, tag="h0")
            nc.gpsimd.dma_start(h0_sb[:], h0_d[:])

            prev_h = [None] * MH
            for c in range(NT):
                if c == 0:
                    xs = xs0
                else:
                    xs = []
                    for kd in range(KD):
                        xt = xpool.tile([128, TC], f32r, tag=f"x{kd}", name=f"x{c}_{kd}")
                        nc.sync.dma_start(xt[:], xT[kd * 128:(kd + 1) * 128, c * TC:(c + 1) * TC])
                        xs.append(xt)
                for j in range(MH):
                    ps = {}
                    for p in "fih":
                        pt = pspool.tile([128, TC], f32, tag="ps", name=f"ps{c}_{j}_{p}")
                        for kd in range(KD):
                            nc.tensor.matmul(
                                pt[:],
                                w_sb[p][kd][:, j * 128:(j + 1) * 128],
                                xs[kd][:],
                                start=(kd == 0),
                                stop=(kd == KD - 1),
                            )
                        ps[p] = pt
                    # Division-free gates via Exp/Ln (single ACT table):
                    # with Ef = e^{-zf}, Ei = e^{-zi}:  f/(f+i) = (1+Ei)/(2+Ef+Ei)
                    # and 1/s2 = Exp(-Ln(s2)); avoids the 3.3us DVE RECIPROCAL.
                    ef = wk.tile([128, TC], f32, tag="ef")
                    nc.scalar.activation(ef[:], ps["f"][:], Act.Exp, bias=b_sb["f"][:, j:j + 1], scale=-1.0)
                    ei = wk.tile([128, TC], f32, tag="ei")
                    nc.scalar.activation(ei[:], ps["i"][:], Act.Exp, bias=b_sb["i"][:, j:j + 1], scale=-1.0)
                    s2 = wk.tile([128, TC], f32, tag="s2")
                    nc.vector.scalar_tensor_tensor(s2[:], ef[:], 2.0, ei[:], A.add, A.add)
                    ln2 = wk.tile([128, TC], f32, tag="ln2")
                    nc.scalar.activation(ln2[:], s2[:], Act.Ln, bias=0.0, scale=1.0)
                    rt = wk.tile([128, TC], f32, tag="rt")
                    nc.scalar.activation(rt[:], ln2[:], Act.Exp, bias=0.0, scale=-1.0)
                    at = wk.tile([128, TC], f32, tag="a")
                    nc.vector.scalar_tensor_tensor(at[:], ei[:], 1.0, rt[:], A.add, A.mult)
                    ut = wk.tile([128, TC], f32, tag="u")
                    nc.vector.scalar_tensor_tensor(ut[:], ef[:], 1.0, rt[:], A.add, A.mult)
                    bt = wk.tile([128, TC], f32, tag="b")
                    nc.vector.scalar_tensor_tensor(bt[:], ps["h"][:], b_sb["h"][:, j:j + 1], ut[:], A.add, A.mult)
                    hh = hpool.tile([128, TC], f32, tag=f"h{j}")
                    init = h0_sb[:, j:j + 1] if c == 0 else prev_h[j][:, TC - 1:TC]
                    nc.vector.tensor_tensor_scan(hh[:], at[:], bt[:], init, op0=A.mult, op1=A.add)
                    prev_h[j] = hh
                    nc.gpsimd.dma_start(hT[j * 128:(j + 1) * 128, c * TC:(c + 1) * TC], hh[:])

    # All our ACT funcs (Exp, Ln, Identity, Copy) live in the single table
    # "natural_log_exp_and_others", but the table-load pass picks the FIRST
    # table containing each func, thrashing Exp->exp_and_others /
    # Ln->natural_log (96 swaps x 1.3us). Empty out every other table (names
    # and positions preserved, so emitted runtime table ids stay valid) so
    # first-match lands on the one shared table and a single load is emitted.
    import concourse.bacc as bacc_mod

    orig_tables = bacc_mod.get_activation_tables

    def _single_table(arch):
        tabs = orig_tables(arch)
        keep = "natural_log_exp_and_others"
        return {k: (v if k == keep else set()) for k, v in tabs.items()}

    bacc_mod.get_activation_tables = _single_table
    try:
        nc.compile()
    finally:
        bacc_mod.get_activation_tables = orig_tables
    return nc


def _get_nc():
    if "nc" not in _state:
        _state["nc"] = _build()
    return _state["nc"]


def _prep_inputs(x, h0, f_w, f_b, i_w, i_b, h_w, h_b):
    x = np.asarray(x, dtype=np.float32)
    h0 = np.asarray(h0, dtype=np.float32)
    xT = np.ascontiguousarray(x.transpose(0, 2, 1))          # [B, D, T]
    shared = {}
    for p, w, bias, sgn in (("f", f_w, f_b, -1.0), ("i", i_w, i_b, -1.0), ("h", h_w, h_b, 1.0)):
        w = np.asarray(w, dtype=np.float32)
        # f/i biases negated: kernel computes Exp(-pre + bias_ap), needs bias_ap = -b
        bias = sgn * np.asarray(bias, dtype=np.float32)
        shared[f"w{p}"] = np.ascontiguousarray(w.T)           # [D, H]
        shared[f"b{p}"] = np.ascontiguousarray(bias.reshape(MH, 128).T)  # [128, MH]
    in_maps = []
    for b in range(B):
        m = dict(shared)
        m["xT"] = xT[b]
        m["h0c"] = np.ascontiguousarray(h0[b, 0].reshape(MH, 128).T)
        in_maps.append(m)
    return in_maps


def kernel(x, h0, f_w, f_b, i_w, i_b, h_w, h_b, _trace=False):
    from concourse.bass_utils import run_bass_kernel_spmd

    nc = _get_nc()
    in_maps = _prep_inputs(x, h0, f_w, f_b, i_w, i_b, h_w, h_b)
    res = run_bass_kernel_spmd(nc, in_maps, core_ids=list(range(B)), trace=_trace)
    out = np.empty((B, T, H), dtype=np.float32)
    for b in range(B):
        out[b] = res.results[b]["hT"].T
    if _trace:
        _state["last_results"] = res
    return out


# revision 18
# speedup vs baseline: 1.0347x; 1.0347x over previous
"""MinLSTM cell for Trainium2 (Bass/Tile), data-parallel over batch on 8 cores.

Per core (one batch row):
  - xT [D,T] layout in SBUF; three projections computed as W^T.T @ xT -> [H,T]
    with fp32r matmuls (full PE rate, ~1.5e-4 rel err) accumulating K=768 in PSUM.
  - division-free gates: with Ef = e^{-zf}, Ei = e^{-zi} (ScalarE Exp straight
    from PSUM, bias fused), a = f/(f+i) = (1+Ei)/(2+Ef+Ei) and the reciprocal
    of s2 = 2+Ef+Ei is Exp(-Ln(s2)) — all ACT funcs from one LUT table.
  - the linear recurrence h_t = a_t*h_{t-1} + b_t runs as one VectorE
    tensor_tensor_scan per [128,TC] tile along the free (time) dim, chained
    across T-chunks via the last column of the previous chunk's output.
  - output written as hT [H,T]; host transposes back to [T,H].
"""

import sys

if "/opt/trn_rl_repo" not in sys.path:
    sys.path.insert(0, "/opt/trn_rl_repo")

import numpy as np

B, T, D, H = 8, 4096, 768, 768
TC = 512                    # time-chunk (one PSUM bank of fp32)
NT = T // TC                # 8 chunks
KD = D // 128               # 6 contraction tiles
MH = H // 128               # 6 hidden tiles

_state = {}


def _build():
    import concourse.mybir as mybir
    import concourse.tile as tile
    from concourse import bacc

    f32, f32r = mybir.dt.float32, mybir.dt.float32r
    A = mybir.AluOpType
    Act = mybir.ActivationFunctionType

    nc = bacc.Bacc("TRN2", target_bir_lowering=False, debug=False, num_devices=B)

    xT = nc.dram_tensor("xT", [D, T], f32r, kind="ExternalInput")
    w_d = {p: nc.dram_tensor(f"w{p}", [D, H], f32r, kind="ExternalInput") for p in "fih"}
    b_d = {p: nc.dram_tensor(f"b{p}", [128, MH], f32, kind="ExternalInput") for p in "fih"}
    h0_d = nc.dram_tensor("h0c", [128, MH], f32, kind="ExternalInput")
    hT = nc.dram_tensor("hT", [H, T], f32, kind="ExternalOutput")

    with tile.TileContext(nc) as tc:
        with (
            tc.tile_pool(name="wpool", bufs=1) as wpool,
            tc.tile_pool(name="cpool", bufs=1) as cpool,
            tc.tile_pool(name="xpool", bufs=2) as xpool,
            tc.tile_pool(name="pspool", bufs=8, space="PSUM") as pspool,
            tc.tile_pool(name="wk", bufs=4) as wk,
            tc.tile_pool(name="hpool", bufs=3) as hpool,
        ):
            # Chunk-0 x first on sync, then each projection's weights on its
            # own issue queue so all three arrive in parallel (~7us head).
            xs0 = []
            for kd in range(KD):
                xt = xpool.tile([128, TC], f32r, tag=f"x{kd}", name=f"x0_{kd}")
                nc.sync.dma_start(xt[:], xT[kd * 128:(kd + 1) * 128, 0:TC])
                xs0.append(xt)
            w_q = {"f": nc.gpsimd, "i": nc.scalar, "h": nc.sync}
            w_sb = {p: [] for p in "fih"}
            for p in "fih":
                for kd in range(KD):
                    t = wpool.tile([128, H], f32r, tag=f"w{p}{kd}", name=f"w{p}{kd}")
                    w_q[p].dma_start(t[:], w_d[p][kd * 128:(kd + 1) * 128, :])
                    w_sb[p].append(t)
            b_sb = {}
            for p in "fih":
                b_sb[p] = cpool.tile([128, MH], f32, tag=f"b{p}", name=f"bs{p}")
                nc.gpsimd.dma_start(b_sb[p][:], b_d[p][:])
            h0_sb = cpool.tile([128, MH], f32, tag="h0")
            nc.gpsimd.dma_start(h0_sb[:], h0_d[:])

            prev_h = [None] * MH
            for c in range(NT):
                if c == 0:
                    xs = xs0
                else:
                    xs = []
                    for kd in range(KD):
                        xt = xpool.tile([128, TC], f32r, tag=f"x{kd}", name=f"x{c}_{kd}")
                        nc.sync.dma_start(xt[:], xT[kd * 128:(kd + 1) * 128, c * TC:(c + 1) * TC])
                        xs.append(xt)
                def emit_group(p, j, ps):
                    pt = pspool.tile([128, TC], f32, tag="ps", name=f"ps{c}_{j}_{p}")
                    for kd in range(KD):
                        nc.tensor.matmul(
                            pt[:],
                            w_sb[p][kd][:, j * 128:(j + 1) * 128],
                            xs[kd][:],
                            start=(kd == 0),
                            stop=(kd == KD - 1),
                        )
                    ps[p] = pt

                # Chunk 0: emit p-major so the PE streams all f-groups while
                # wi/wh weight DMAs are still in flight (wf arrives first).
                ps_by_j = [dict() for _ in range(MH)]
                if c == 0:
                    for p in "fih":
                        for j in range(MH):
                            emit_group(p, j, ps_by_j[j])
                for j in range(MH):
                    ps = ps_by_j[j]
                    if c != 0:
                        for p in "fih":
                            emit_group(p, j, ps)
                    # Division-free gates via Exp/Ln (single ACT table):
                    # with Ef = e^{-zf}, Ei = e^{-zi}:  f/(f+i) = (1+Ei)/(2+Ef+Ei)
                    # and 1/s2 = Exp(-Ln(s2)); avoids the 3.3us DVE RECIPROCAL.
                    ef = wk.tile([128, TC], f32, tag="ef")
                    nc.scalar.activation(ef[:], ps["f"][:], Act.Exp, bias=b_sb["f"][:, j:j + 1], scale=-1.0)
                    ei = wk.tile([128, TC], f32, tag="ei")
                    nc.scalar.activation(ei[:], ps["i"][:], Act.Exp, bias=b_sb["i"][:, j:j + 1], scale=-1.0)
                    s2 = wk.tile([128, TC], f32, tag="s2")
                    nc.vector.scalar_tensor_tensor(s2[:], ef[:], 2.0, ei[:], A.add, A.add)
                    ln2 = wk.tile([128, TC], f32, tag="ln2")
                    nc.scalar.activation(ln2[:], s2[:], Act.Ln, bias=0.0, scale=1.0)
                    rt = wk.tile([128, TC], f32, tag="rt")
                    nc.scalar.activation(rt[:], ln2[:], Act.Exp, bias=0.0, scale=-1.0)
                    at = wk.tile([128, TC], f32, tag="a")
                    nc.vector.scalar_tensor_tensor(at[:], ei[:], 1.0, rt[:], A.add, A.mult)
                    ut = wk.tile([128, TC], f32, tag="u")
                    nc.vector.scalar_tensor_tensor(ut[:], ef[:], 1.0, rt[:], A.add, A.mult)
                    bt = wk.tile([128, TC], f32, tag="b")
                    nc.vector.scalar_tensor_tensor(bt[:], ps["h"][:], b_sb["h"][:, j:j + 1], ut[:], A.add, A.mult)
                    hh = hpool.tile([128, TC], f32, tag=f"h{j}")
                    init = h0_sb[:, j:j + 1] if c == 0 else prev_h[j][:, TC - 1:TC]
                    nc.vector.tensor_tensor_scan(hh[:], at[:], bt[:], init, op0=A.mult, op1=A.add)
                    prev_h[j] = hh
                    nc.gpsimd.dma_start(hT[j * 128:(j + 1) * 128, c * TC:(c + 1) * TC], hh[:])

    # All our ACT funcs (Exp, Ln, Identity, Copy) live in the single table
    # "natural_log_exp_and_others", but the table-load pass picks the FIRST
    # table containing each func, thrashing Exp->exp_and_others /
    # Ln->natural_log (96 swaps x 1.3us). Empty out every other table (names
    # and positions preserved, so emitted runtime table ids stay valid) so
    # first-match lands on the one shared table and a single load is emitted.
    import concourse.bacc as bacc_mod

    orig_tables = bacc_mod.get_activation_tables

    def _single_table(arch):
        tabs = orig_tables(arch)
        keep = "natural_log_exp_and_others"
        return {k: (v if k == keep else set()) for k, v in tabs.items()}

    bacc_mod.get_activation_tables = _single_table
    try:
        nc.compile()
    finally:
        bacc_mod.get_activation_tables = orig_tables
    return nc


def _get_nc():
    if "nc" not in _state:
        _state["nc"] = _build()
    return _state["nc"]


def _prep_inputs(x, h0, f_w, f_b, i_w, i_b, h_w, h_b):
    x = np.asarray(x, dtype=np.float32)
    h0 = np.asarray(h0, dtype=np.float32)
    xT = np.ascontiguousarray(x.transpose(0, 2, 1))          # [B, D, T]
    shared = {}
    for p, w, bias, sgn in (("f", f_w, f_b, -1.0), ("i", i_w, i_b, -1.0), ("h", h_w, h_b, 1.0)):
        w = np.asarray(w, dtype=np.float32)
        # f/i biases negated: kernel computes Exp(-pre + bias_ap), needs bias_ap = -b
        bias = sgn * np.asarray(bias, dtype=np.float32)
        shared[f"w{p}"] = np.ascontiguousarray(w.T)           # [D, H]
        shared[f"b{p}"] = np.ascontiguousarray(bias.reshape(MH, 128).T)  # [128, MH]
    in_maps = []
    for b in range(B):
        m = dict(shared)
        m["xT"] = xT[b]
        m["h0c"] = np.ascontiguousarray(h0[b, 0].reshape(MH, 128).T)
        in_maps.append(m)
    return in_maps


def kernel(x, h0, f_w, f_b, i_w, i_b, h_w, h_b, _trace=False):
    from concourse.bass_utils import run_bass_kernel_spmd

    nc = _get_nc()
    in_maps = _prep_inputs(x, h0, f_w, f_b, i_w, i_b, h_w, h_b)
    res = run_bass_kernel_spmd(nc, in_maps, core_ids=list(range(B)), trace=_trace)
    out = np.empty((B, T, H), dtype=np.float32)
    for b in range(B):
        out[b] = res.results[b]["hT"].T
    if _trace:
        _state["last_results"] = res
    return out
